# revision 19
# baseline (speedup 1.0000x reference)
"""Trainium2 Bass kernel for nn_DegModel (blind-SR degradation model).

Sharding: pure data parallel, 1 image per NeuronCore (B=8 over 8 cores).
BatchNorm batch statistics are made exact via per-layer [64,2] AllReduces.
KernelModel (1x1-conv MLP on (8,64) latents, ~1 MFLOP) runs on host in fp32.
"""
import numpy as np
import ml_dtypes

import concourse.bass as bass
import concourse.bacc as bacc
import concourse.mybir as mybir
import concourse.tile as tile
from concourse.bass_utils import run_bass_kernel_spmd

F32 = mybir.dt.float32
BF16 = mybir.dt.bfloat16
AX = mybir.AxisListType
OP = mybir.AluOpType
ACTF = mybir.ActivationFunctionType

B, C, H, W = 8, 3, 512, 512
SCALE, K = 4, 21
NF, NB = 8 * 8, 8  # 64 channels, 8 blocks
h, w = H // SCALE, W // SCALE  # 128, 128
EPS = 1e-5
PAD = K // 2  # 10
N_CORES = 8
N_GLOBAL = float(B * h * w)  # BN count over batch+spatial
ROWT = 108  # blur row-tile stride (128-row tiles, 20-row overlap)
NT = 5      # blur row tiles

bf = ml_dtypes.bfloat16


# ----------------------------------------------------------------- host math
def _np(x):
    return np.asarray(x, dtype=np.float32)


def _kernel_model_host(zk, kp):
    """Exact fp32 KernelModel: 1x1 convs + batch BN + softmax -> (B, K*K)."""
    x = _np(zk)[:, :, 0, 0]

    def conv(xx, wt, b):
        return xx @ _np(wt)[:, :, 0, 0].T + _np(b)

    def bn(xx, g, be):
        m = xx.mean(0, keepdims=True)
        v = xx.var(0, keepdims=True)
        return _np(g)[None] * (xx - m) / np.sqrt(v + EPS) + _np(be)[None]

    y = np.maximum(bn(conv(x, kp["hw"], kp["hb"]), kp["hg"], kp["hbe"]), 0)
    for blk in kp["blocks"]:
        r = np.maximum(bn(conv(y, blk["w1"], blk["b1"]), blk["g"], blk["be"]), 0)
        r = conv(r, blk["w2"], blk["b2"])
        y = y + r
    klog = conv(y, kp["tw"], kp["tb"]).astype(np.float64)
    e = np.exp(klog - klog.max(1, keepdims=True))
    return (e / e.sum(1, keepdims=True)).astype(np.float32)


def _shift2(img130):
    """flat shift by -2: out.flat[i] = in.flat[i+2]."""
    out = np.zeros_like(img130)
    out.flat[: img130.size - 2] = img130.flat[2:]
    return out


def _pack_core_inputs(b, inp, zn, kvec, npar):
    """Build the per-core input arrays (host-side layout/pack, all cheap)."""
    ins = {}
    # --- blur image: reflect pad then banded row tiles (128 rows, stride 108)
    xp = np.pad(_np(inp)[b], ((0, 0), (PAD, PAD), (PAD, PAD)), mode="reflect")
    xp_sb = np.zeros((128, C, NT, W + 2 * PAD), dtype=bf)
    for t in range(NT):
        r0 = ROWT * t
        n = min(128, xp.shape[1] - r0)
        xp_sb[:n, :, t, :] = xp[:, r0 : r0 + n, :].transpose(1, 0, 2).astype(bf)
    ins["xp_sb"] = xp_sb.reshape(128, -1)

    # --- blur band weights from kvec: wband[kx][p, di] = kvec[p-4di, kx]
    kv = kvec[b].reshape(K, K)
    wband = np.zeros((128, K, 27), dtype=np.float32)
    for kx in range(K):
        for di in range(27):
            for ky in range(K):
                p = 4 * di + ky
                if p < 128:
                    wband[p, kx, di] = kv[ky, kx]
    ins["wband"] = wband.reshape(128, -1).astype(bf)

    # --- head input prepack: zeros + zn at partitions 3..10 (and shifted copy
    #     at partitions 67..74), 130x130 zero-padded interior layout
    head = np.zeros((128, 130, 130), dtype=bf)
    znb = _np(zn)[b]  # (8, 128, 128)
    for k in range(8):
        z = np.zeros((130, 130), dtype=np.float32)
        z[1:129, 1:129] = znb[k]
        zb = z.astype(bf)
        head[3 + k] = zb
        head[64 + 3 + k] = _shift2(zb)
    ins["head_in"] = head.reshape(128, -1)

    # --- conv weights: 6 passes (3 pairs dy=-1..1 with dx=-1/+1, 3 singles dx=0)
    def pack6(wt, cin):
        out = np.zeros((128, 6, 64), dtype=bf)
        wt = _np(wt)  # (64, cin, 3, 3)
        for p in range(3):  # pairs
            out[:cin, p, :] = wt[:, :, p, 0].T.astype(bf)
            out[64 : 64 + cin, p, :] = wt[:, :, p, 2].T.astype(bf)
        for p in range(3):  # singles
            out[:cin, 3 + p, :] = wt[:, :, p, 1].T.astype(bf)
        return out

    ins["w_head"] = pack6(npar["hw"], 3 + 8).reshape(128, -1)
    wblk = np.zeros((128, 16, 6, 64), dtype=bf)
    for i, blk in enumerate(npar["blocks"]):
        wblk[:, 2 * i] = pack6(blk["w1"], 64)
        wblk[:, 2 * i + 1] = pack6(blk["w2"], 64)
    ins["w_blk"] = wblk.reshape(128, -1)
    wtail = np.zeros((128, 3), dtype=bf)
    wtail[:64, :] = _np(npar["tw"])[:, :, 0, 0].T.astype(bf)
    ins["w_tail"] = wtail

    # --- consts: col 0-7 = b2 per block, col 8 = tail bias (parts 0-2),
    #     col 9 = eps
    cb = np.zeros((64, 16), dtype=np.float32)
    for i, blk in enumerate(npar["blocks"]):
        cb[:, i] = _np(blk["b2"])
    cb[:3, 8] = _np(npar["tb"])
    cb[:, 9] = EPS
    ins["cb"] = cb

    # --- BN gamma/beta per layer (0=head, 1..8=blocks conv1)
    bn = np.zeros((64, 18), dtype=np.float32)
    bn[:, 0] = _np(npar["hg"])
    bn[:, 1] = _np(npar["hbe"])
    for i, blk in enumerate(npar["blocks"]):
        bn[:, 2 + 2 * i] = _np(blk["g"])
        bn[:, 3 + 2 * i] = _np(blk["be"])
    ins["bn"] = bn
    return ins


# -------------------------------------------------------------- device build
def _shifted_dst(tile3):
    """[64,128,128] AP over partitions 64-127 at interior flat offset -2."""
    flat = tile3.rearrange("p a b -> p (a b)")
    s = flat[64:128, 129 : 129 + 128 * 130]
    return s.rearrange("p (a b) -> p a b", b=130)[:, :, 0:128]


def _build(nc):
    d_xp = nc.dram_tensor("xp_sb", [128, C * NT * 532], BF16, kind="ExternalInput")
    d_wband = nc.dram_tensor("wband", [128, K * 27], BF16, kind="ExternalInput")
    d_head = nc.dram_tensor("head_in", [128, 130 * 130], BF16, kind="ExternalInput")
    d_whead = nc.dram_tensor("w_head", [128, 6 * 64], BF16, kind="ExternalInput")
    d_wblk = nc.dram_tensor("w_blk", [128, 16 * 6 * 64], BF16, kind="ExternalInput")
    d_wtail = nc.dram_tensor("w_tail", [128, 3], BF16, kind="ExternalInput")
    d_cb = nc.dram_tensor("cb", [64, 16], F32, kind="ExternalInput")
    d_bn = nc.dram_tensor("bn", [64, 18], F32, kind="ExternalInput")
    d_deg = nc.dram_tensor("deg", [C, h, w], F32, kind="ExternalOutput")
    d_noise = nc.dram_tensor("noise", [C, h, w], F32, kind="ExternalOutput")

    rg = [list(range(N_CORES))]

    # ---- persistent SBUF tensors (static addresses, outside the Tile pools)
    def T(name, shape, dtype):
        return nc.alloc_sbuf_tensor(name, list(shape), dtype).ap()

    wblk_sb = T("wblk_sb", [128, 16, 6, 64], BF16)
    whead_sb = T("whead_sb", [128, 6, 64], BF16)
    wtail_sb = T("wtail_sb", [128, 3], BF16)
    cb_sb = T("cb_sb", [64, 16], F32)
    bn_sb = T("bn_sb", [64, 18], F32)
    craw = T("craw", [64, h, w], BF16)
    y_master = T("y_master", [64, h, w], F32)
    bnst = T("bnst", [64, 32, 6], F32)
    mvloc = T("mvloc", [64, 2], F32)
    comm = T("comm", [64, 4], F32)
    stats = comm[:, 0:2]
    gstats = comm[:, 2:4]
    smalls = T("smalls", [64, 8], F32)
    mean = smalls[:, 0:1]
    msq = smalls[:, 1:2]
    var = smalls[:, 2:3]
    sv = smalls[:, 3:4]
    rs = smalls[:, 4:5]
    scale_t = smalls[:, 5:6]
    mt = smalls[:, 6:7]
    tbias = smalls[:, 7:8]
    eps_ap = cb_sb[:, 9:10]
    # padded conv activation buffers; head_in aliases r_pad (disjoint lifetime)
    y_pad = T("y_pad", [128, 130, 130], BF16)
    r_pad = T("r_pad", [128, 130, 130], BF16)
    head_in = r_pad

    from contextlib import ExitStack
    est = ExitStack()
    with tile.TileContext(nc) as tc, est:
        dram = est.enter_context(tc.tile_pool(name="dram", bufs=1, space="DRAM"))
        # DRAM scratch for x_lr (row-major fp32 for output add, bf16 for head)
        xlr32 = dram.tile([h, C, w], F32, name="xlr32")
        xlr16 = dram.tile([C, h, w], BF16, name="xlr16")

        wbf = wblk_sb.rearrange("p a b c -> p (a b c)")
        for q in range(4):
            nc.sync.dma_start(wbf[32 * q : 32 * q + 32, :],
                              d_wblk.ap()[32 * q : 32 * q + 32, :])
        nc.sync.dma_start(whead_sb.rearrange("p a b -> p (a b)"), d_whead.ap())
        nc.sync.dma_start(wtail_sb[:], d_wtail.ap())
        nc.sync.dma_start(cb_sb[:], d_cb.ap())
        nc.sync.dma_start(bn_sb[:], d_bn.ap())
        # head_in zn/zero prepack (must precede the blur-time x_lr fills)
        hflat = head_in.rearrange("p a b -> p (a b)")
        for q in range(8):
            nc.sync.dma_start(hflat[16 * q : 16 * q + 16, :],
                              d_head.ap()[16 * q : 16 * q + 16, :])

        # ================= phase 1: blur =================
        with tc.tile_pool(name="blur", bufs=1) as bp, \
             tc.tile_pool(name="blur_xp", bufs=3) as bxp, \
             tc.tile_pool(name="blur_ps", bufs=3, space="PSUM") as bps:
            wband_sb = bp.tile([128, K, 27], BF16, name="wband_sb")
            nc.sync.dma_start(wband_sb.rearrange("p a b -> p (a b)"), d_wband.ap())
            d_xp3 = d_xp.ap().rearrange("p (c t x) -> p c t x", c=C, t=NT)
            for t in range(NT):
                nt = 27 if t < 4 else 20
                xp_t = bxp.tile([128, C, 532], BF16, name="xp_t", tag="xpt")
                for q in range(8):
                    nc.sync.dma_start(xp_t[16 * q : 16 * q + 16],
                                      d_xp3[16 * q : 16 * q + 16, :, t, :])
                ps = bps.tile([27, C, w], F32, name="blur_ps_t", tag="bps")
                for kx in range(K):
                    nc.tensor.matmul(
                        ps[:, :, :],
                        lhsT=wband_sb[:, kx, :],
                        rhs=xp_t[:, :, kx : kx + 4 * w : 4],
                        start=(kx == 0),
                        stop=(kx == K - 1),
                    )
                tmp32 = bp.tile([27, C, w], F32, name="tmp32", tag="t32", bufs=2)
                tmp16 = bp.tile([27, C, w], BF16, name="tmp16", tag="t16", bufs=2)
                nc.scalar.copy(tmp32[:nt], ps[:nt])
                nc.vector.tensor_copy(tmp16[:nt], ps[:nt])
                nc.sync.dma_start(xlr32[ROWT // 4 * t : ROWT // 4 * t + nt], tmp32[:nt])
                for c in range(C):
                    nc.sync.dma_start(
                        xlr16[c, 27 * t : 27 * t + nt, :], tmp16[:nt, c, :]
                    )
                # fill head_in x_lr rows for this tile (base + shifted copy)
                r0, r1 = 27 * t, 27 * t + nt
                nc.sync.dma_start(head_in[0:3, 1 + r0 : 1 + r1, 1:129],
                                  xlr16[:, r0:r1, :])
                nc.sync.dma_start(_shifted_dst(head_in)[0:3, r0:r1, :],
                                  xlr16[:, r0:r1, :])

        # ================= phase 2+3: head conv + resblocks =================
        def conv_pass(in3, wsrc, psum_pool, evac):
            """One conv layer: 6 tap passes over 16 chunk-pairs, col-tiled x2.

            in3: [128,130,130] padded input (parts 64-127 = flat-2 copy)
            wsrc: [*, 6, 64] weight slices; evac(cp, ps) consumes psum tile
            [128, 4, 128]x2 (chunk 2cp rows in parts 0-63, 2cp+1 in 64-127).
            """
            for cp in range(16):
                rA, rB = 8 * cp, 8 * cp + 4
                ps = psum_pool.tile([128, 4, w], F32, name="cps", tag="cps")
                for p in range(6):
                    dy = p % 3 - 1
                    dx = -1 if p < 3 else 0
                    lhsT = wsrc[:, p, :]
                    nc.tensor.matmul(
                        ps[0:64], lhsT=lhsT,
                        rhs=in3[:, rA + 1 + dy : rA + 5 + dy, 1 + dx : 129 + dx],
                        start=(p == 0), stop=(p == 5),
                    )
                    nc.tensor.matmul(
                        ps[64:128], lhsT=lhsT,
                        rhs=in3[:, rB + 1 + dy : rB + 5 + dy, 1 + dx : 129 + dx],
                        start=(p == 0), stop=(p == 5),
                    )
                evac(cp, ps)

        def evac_stats(cp, ps):
            """conv1/head evac: raw -> craw (bf16) + per-chunk bn_stats."""
            for half, r0 in ((0, 8 * cp), (1, 8 * cp + 4)):
                pslc = ps[64 * half : 64 * half + 64]
                nc.scalar.activation(craw[:, r0 : r0 + 4, :], pslc, ACTF.Copy)
                nc.vector.bn_stats(
                    bnst[:, 2 * cp + half, :],
                    craw[:, r0 : r0 + 4, :].rearrange("p a b -> p (a b)"))

        def bn_allreduce(layer):
            """Reduce chunk stats, AllReduce, compute scale_t/tbias."""
            nc.vector.bn_aggr(mvloc[:], bnst.rearrange("p a b -> p (a b)"))
            # local (mean,var) -> (sum, sumsq) for cross-core reduction (all DVE)
            nc.vector.tensor_scalar_mul(stats[:, 0:1], mvloc[:, 0:1], float(h * w))
            nc.vector.tensor_mul(msq, mvloc[:, 0:1], mvloc[:, 0:1])
            nc.vector.tensor_add(sv, mvloc[:, 1:2], msq)
            nc.vector.tensor_scalar_mul(stats[:, 1:2], sv, float(h * w))
            cc_in = dram.tile([64, 2], F32, name=f"cc_in_{layer}")
            cc_out = dram.tile([64, 2], F32, name=f"cc_out_{layer}",
                               addr_space="Shared")
            nc.sync.dma_start(cc_in[:], stats)
            nc.gpsimd.collective_compute(
                "AllReduce", OP.add, replica_groups=rg,
                ins=[cc_in.opt()], outs=[cc_out.opt()],
            )
            nc.sync.dma_start(gstats, cc_out[:])
            # smalls[:,0:2] = (mean, qn) = gstats / N  (one DVE op)
            nc.vector.tensor_scalar_mul(smalls[:, 0:2], gstats, 1.0 / N_GLOBAL)
            # nvar = mean^2 - qn  (= -var); sv = sqrt(-nvar + eps)
            nc.vector.scalar_tensor_tensor(var, mean, mean, msq,
                                           op0=OP.mult, op1=OP.subtract)
            nc.scalar.activation(sv, var, ACTF.Sqrt, bias=eps_ap, scale=-1.0)
            nc.vector.reciprocal(rs, sv)
            nc.vector.tensor_mul(scale_t, rs, bn_sb[:, 2 * layer : 2 * layer + 1])
            nc.vector.tensor_mul(mt, mean, scale_t)
            nc.vector.tensor_sub(tbias, bn_sb[:, 2 * layer + 1 : 2 * layer + 2], mt)

        # ---- head conv (head_in aliases r_pad storage)
        with tc.tile_pool(name="conv_ps", bufs=8, space="PSUM") as cps_pool:
            conv_pass(head_in, whead_sb, cps_pool, evac_stats)
            bn_allreduce(0)

            # y0 = relu(bn(head)): fp32 master + bf16 padded (base + shifted)
            for buf in (y_pad, r_pad):
                nc.vector.memset(buf[:, 0, :], 0.0)      # top row
                nc.vector.memset(buf[:, 129, :], 0.0)    # bottom row
                nc.vector.memset(buf[:, 1:129, 0:1], 0.0)    # left col
                nc.vector.memset(buf[:, 1:129, 127:130], 0.0)  # right cols+guard
            for j, n in [(0, 12)] + [(12 + 16 * i, 16) for i in range(7)] + [(124, 4)]:
                nc.scalar.activation(y_pad[0:64, j + 1 : j + 1 + n, 1:129],
                                     craw[:, j : j + n, :],
                                     ACTF.Relu, bias=tbias, scale=scale_t)
                nc.vector.tensor_copy(_shifted_dst(y_pad)[:, j : j + n, :],
                                      y_pad[0:64, j + 1 : j + 1 + n, 1:129])

            # ---- 8 resblocks
            for blk in range(NB):
                conv_pass(y_pad, wblk_sb[:, 2 * blk], cps_pool, evac_stats)
                bn_allreduce(1 + blk)
                for j, n in [(0, 12)] + [(12 + 16 * i, 16) for i in range(7)] + [(124, 4)]:
                    nc.scalar.activation(r_pad[0:64, j + 1 : j + 1 + n, 1:129],
                                         craw[:, j : j + n, :],
                                         ACTF.Relu, bias=tbias, scale=scale_t)
                    nc.vector.tensor_copy(
                        _shifted_dst(r_pad)[:, j : j + n, :],
                        r_pad[0:64, j + 1 : j + 1 + n, 1:129])

                b2_ap = cb_sb[:, blk : blk + 1]

                def evac_resid(cp, ps, b2_ap=b2_ap, first=(blk == 0)):
                    for half, r0 in ((0, 8 * cp), (1, 8 * cp + 4)):
                        pslc = ps[64 * half : 64 * half + 64]
                        # block 0 reads y0 from the bf16 padded buffer (no
                        # separate fp32 y0 materialization needed)
                        prev = (y_pad[0:64, r0 + 1 : r0 + 5, 1:129] if first
                                else y_master[:, r0 : r0 + 4, :])
                        nc.vector.scalar_tensor_tensor(
                            y_master[:, r0 : r0 + 4, :], pslc, b2_ap,
                            prev, op0=OP.add, op1=OP.add,
                        )

                conv_pass(r_pad, wblk_sb[:, 2 * blk + 1], cps_pool, evac_resid)
                chks = [(0, 12)] + [(12 + 16 * i, 16) for i in range(7)] + [(124, 4)]
                for ci, (j, n) in enumerate(chks):
                    if ci % 2 == 0:
                        nc.scalar.copy(y_pad[0:64, j + 1 : j + 1 + n, 1:129],
                                       y_master[:, j : j + n, :])
                    else:
                        nc.vector.tensor_copy(
                            y_pad[0:64, j + 1 : j + 1 + n, 1:129],
                            y_master[:, j : j + n, :])
                    nc.vector.tensor_copy(
                        _shifted_dst(y_pad)[:, j : j + n, :],
                        y_pad[0:64, j + 1 : j + 1 + n, 1:129])

            # ---- tail 1x1 conv + outputs
            tb_ap = cb_sb[:, 8:9]
            # y_master is dead after the last y_pad rebuild; reuse its
            # partitions 0-2 as the x_lr fp32 staging buffer for the add
            xlr_sb = y_master[0:3, :, :]
            xlr_cij = xlr32.rearrange("i c j -> c i j")
            for c in range(C):
                nc.sync.dma_start(xlr_sb[c : c + 1], xlr_cij[c : c + 1])
            with tc.tile_pool(name="tailp", bufs=4) as tp:
                for cp in range(16):
                    ps = cps_pool.tile([128, 4, w], F32, name="tps", tag="cps")
                    for half, r0 in ((0, 8 * cp), (1, 8 * cp + 4)):
                        nc.tensor.matmul(
                            ps[64 * half : 64 * half + 3],
                            lhsT=wtail_sb[0:64, :],
                            rhs=y_pad[0:64, r0 + 1 : r0 + 5, 1:129],
                            start=True, stop=True,
                        )
                    for half, r0 in ((0, 8 * cp), (1, 8 * cp + 4)):
                        pslc = ps[64 * half : 64 * half + 3]
                        nstage = tp.tile([3, 4, w], F32, name="nstage", tag="nst")
                        dstage = tp.tile([3, 4, w], F32, name="dstage", tag="dst")
                        nc.scalar.activation(nstage[:], pslc, ACTF.Identity,
                                             bias=tb_ap[0:3], scale=1.0)
                        nc.vector.scalar_tensor_tensor(
                            dstage[:], pslc, tb_ap[0:3],
                            xlr_sb[:, r0 : r0 + 4, :],
                            op0=OP.add, op1=OP.add,
                        )
                        nc.sync.dma_start(d_noise.ap()[:, r0 : r0 + 4, :], nstage[:])
                        nc.sync.dma_start(d_deg.ap()[:, r0 : r0 + 4, :], dstage[:])
    return nc


_CACHE = {}


def _get_program():
    if "nc" not in _CACHE:
        nc = bacc.Bacc("TRN2", target_bir_lowering=False, debug=False,
                       num_devices=N_CORES)
        _build(nc)
        nc.compile()
        _CACHE["nc"] = nc
    return _CACHE["nc"]


def kernel(inp, zk, zn, kparams, nparams):
    kvec = _kernel_model_host(zk, kparams)  # (B, K*K)

    in_maps = [_pack_core_inputs(b, inp, zn, kvec, nparams) for b in range(B)]
    nc = _get_program()
    res = run_bass_kernel_spmd(nc, in_maps, core_ids=list(range(N_CORES)))
    kernel._last_perf = res

    deg = np.stack([res.results[b]["deg"] for b in range(B)])
    noise = np.stack([res.results[b]["noise"] for b in range(B)])
    return (deg.astype(np.float32), kvec.reshape(B, K, K),
            noise.astype(np.float32))


# revision 20
# speedup vs baseline: 1.0000x; 1.0000x over previous
"""Trainium2 Bass kernel for nn_DegModel (blind-SR degradation model).

Sharding: pure data parallel, 1 image per NeuronCore (B=8 over 8 cores).
BatchNorm batch statistics are made exact via per-layer [64,2] AllReduces.
KernelModel (1x1-conv MLP on (8,64) latents, ~1 MFLOP) runs on host in fp32.
"""
import numpy as np
import ml_dtypes

import concourse.bass as bass
import concourse.bacc as bacc
import concourse.mybir as mybir
import concourse.tile as tile
from concourse.bass_utils import run_bass_kernel_spmd

F32 = mybir.dt.float32
BF16 = mybir.dt.bfloat16
AX = mybir.AxisListType
OP = mybir.AluOpType
ACTF = mybir.ActivationFunctionType

B, C, H, W = 8, 3, 512, 512
SCALE, K = 4, 21
NF, NB = 8 * 8, 8  # 64 channels, 8 blocks
h, w = H // SCALE, W // SCALE  # 128, 128
EPS = 1e-5
PAD = K // 2  # 10
N_CORES = 8
N_GLOBAL = float(B * h * w)  # BN count over batch+spatial
ROWT = 108  # blur row-tile stride (128-row tiles, 20-row overlap)
NT = 5      # blur row tiles

bf = ml_dtypes.bfloat16


# ----------------------------------------------------------------- host math
def _np(x):
    return np.asarray(x, dtype=np.float32)


def _kernel_model_host(zk, kp):
    """Exact fp32 KernelModel: 1x1 convs + batch BN + softmax -> (B, K*K)."""
    x = _np(zk)[:, :, 0, 0]

    def conv(xx, wt, b):
        return xx @ _np(wt)[:, :, 0, 0].T + _np(b)

    def bn(xx, g, be):
        m = xx.mean(0, keepdims=True)
        v = xx.var(0, keepdims=True)
        return _np(g)[None] * (xx - m) / np.sqrt(v + EPS) + _np(be)[None]

    y = np.maximum(bn(conv(x, kp["hw"], kp["hb"]), kp["hg"], kp["hbe"]), 0)
    for blk in kp["blocks"]:
        r = np.maximum(bn(conv(y, blk["w1"], blk["b1"]), blk["g"], blk["be"]), 0)
        r = conv(r, blk["w2"], blk["b2"])
        y = y + r
    klog = conv(y, kp["tw"], kp["tb"]).astype(np.float64)
    e = np.exp(klog - klog.max(1, keepdims=True))
    return (e / e.sum(1, keepdims=True)).astype(np.float32)


def _shift2(img130):
    """flat shift by -2: out.flat[i] = in.flat[i+2]."""
    out = np.zeros_like(img130)
    out.flat[: img130.size - 2] = img130.flat[2:]
    return out


def _pack_core_inputs(b, inp, zn, kvec, npar):
    """Build the per-core input arrays (host-side layout/pack, all cheap)."""
    ins = {}
    # --- blur image: reflect pad then banded row tiles (128 rows, stride 108)
    xp = np.pad(_np(inp)[b], ((0, 0), (PAD, PAD), (PAD, PAD)), mode="reflect")
    xp_sb = np.zeros((128, C, NT, W + 2 * PAD), dtype=bf)
    for t in range(NT):
        r0 = ROWT * t
        n = min(128, xp.shape[1] - r0)
        xp_sb[:n, :, t, :] = xp[:, r0 : r0 + n, :].transpose(1, 0, 2).astype(bf)
    ins["xp_sb"] = xp_sb.reshape(128, -1)

    # --- blur band weights from kvec: wband[kx][p, di] = kvec[p-4di, kx]
    kv = kvec[b].reshape(K, K)
    wband = np.zeros((128, K, 27), dtype=np.float32)
    for kx in range(K):
        for di in range(27):
            for ky in range(K):
                p = 4 * di + ky
                if p < 128:
                    wband[p, kx, di] = kv[ky, kx]
    ins["wband"] = wband.reshape(128, -1).astype(bf)

    # --- head input prepack: zeros + zn at partitions 3..10 (and shifted copy
    #     at partitions 67..74), 130x130 zero-padded interior layout
    head = np.zeros((128, 130, 130), dtype=bf)
    znb = _np(zn)[b]  # (8, 128, 128)
    for k in range(8):
        z = np.zeros((130, 130), dtype=np.float32)
        z[1:129, 1:129] = znb[k]
        zb = z.astype(bf)
        head[3 + k] = zb
        head[64 + 3 + k] = _shift2(zb)
    ins["head_in"] = head.reshape(128, -1)

    # --- conv weights: 6 passes (3 pairs dy=-1..1 with dx=-1/+1, 3 singles dx=0)
    def pack6(wt, cin):
        out = np.zeros((128, 6, 64), dtype=bf)
        wt = _np(wt)  # (64, cin, 3, 3)
        for p in range(3):  # pairs
            out[:cin, p, :] = wt[:, :, p, 0].T.astype(bf)
            out[64 : 64 + cin, p, :] = wt[:, :, p, 2].T.astype(bf)
        for p in range(3):  # singles
            out[:cin, 3 + p, :] = wt[:, :, p, 1].T.astype(bf)
        return out

    ins["w_head"] = pack6(npar["hw"], 3 + 8).reshape(128, -1)
    wblk = np.zeros((128, 16, 6, 64), dtype=bf)
    for i, blk in enumerate(npar["blocks"]):
        wblk[:, 2 * i] = pack6(blk["w1"], 64)
        wblk[:, 2 * i + 1] = pack6(blk["w2"], 64)
    ins["w_blk"] = wblk.reshape(128, -1)
    wtail = np.zeros((128, 3), dtype=bf)
    wtail[:64, :] = _np(npar["tw"])[:, :, 0, 0].T.astype(bf)
    ins["w_tail"] = wtail

    # --- consts: col 0-7 = b2 per block, col 8 = tail bias (parts 0-2),
    #     col 9 = eps
    cb = np.zeros((64, 16), dtype=np.float32)
    for i, blk in enumerate(npar["blocks"]):
        cb[:, i] = _np(blk["b2"])
    cb[:3, 8] = _np(npar["tb"])
    cb[:, 9] = EPS
    ins["cb"] = cb

    # --- BN gamma/beta per layer (0=head, 1..8=blocks conv1)
    bn = np.zeros((64, 18), dtype=np.float32)
    bn[:, 0] = _np(npar["hg"])
    bn[:, 1] = _np(npar["hbe"])
    for i, blk in enumerate(npar["blocks"]):
        bn[:, 2 + 2 * i] = _np(blk["g"])
        bn[:, 3 + 2 * i] = _np(blk["be"])
    ins["bn"] = bn
    return ins


# -------------------------------------------------------------- device build
def _shifted_dst(tile3):
    """[64,128,128] AP over partitions 64-127 at interior flat offset -2."""
    flat = tile3.rearrange("p a b -> p (a b)")
    s = flat[64:128, 129 : 129 + 128 * 130]
    return s.rearrange("p (a b) -> p a b", b=130)[:, :, 0:128]


def _build(nc):
    d_xp = nc.dram_tensor("xp_sb", [128, C * NT * 532], BF16, kind="ExternalInput")
    d_wband = nc.dram_tensor("wband", [128, K * 27], BF16, kind="ExternalInput")
    d_head = nc.dram_tensor("head_in", [128, 130 * 130], BF16, kind="ExternalInput")
    d_whead = nc.dram_tensor("w_head", [128, 6 * 64], BF16, kind="ExternalInput")
    d_wblk = nc.dram_tensor("w_blk", [128, 16 * 6 * 64], BF16, kind="ExternalInput")
    d_wtail = nc.dram_tensor("w_tail", [128, 3], BF16, kind="ExternalInput")
    d_cb = nc.dram_tensor("cb", [64, 16], F32, kind="ExternalInput")
    d_bn = nc.dram_tensor("bn", [64, 18], F32, kind="ExternalInput")
    d_deg = nc.dram_tensor("deg", [C, h, w], F32, kind="ExternalOutput")
    d_noise = nc.dram_tensor("noise", [C, h, w], F32, kind="ExternalOutput")

    rg = [list(range(N_CORES))]

    # ---- persistent SBUF tensors (static addresses, outside the Tile pools)
    def T(name, shape, dtype):
        return nc.alloc_sbuf_tensor(name, list(shape), dtype).ap()

    wblk_sb = T("wblk_sb", [128, 16, 6, 64], BF16)
    whead_sb = T("whead_sb", [128, 6, 64], BF16)
    wtail_sb = T("wtail_sb", [128, 3], BF16)
    cb_sb = T("cb_sb", [64, 16], F32)
    bn_sb = T("bn_sb", [64, 18], F32)
    craw = T("craw", [64, h, w], BF16)
    y_master = T("y_master", [64, h, w], F32)
    bnst = T("bnst", [64, 32, 6], F32)
    mvloc = T("mvloc", [64, 2], F32)
    comm = T("comm", [64, 4], F32)
    stats = comm[:, 0:2]
    gstats = comm[:, 2:4]
    smalls = T("smalls", [64, 8], F32)
    mean = smalls[:, 0:1]
    msq = smalls[:, 1:2]
    var = smalls[:, 2:3]
    sv = smalls[:, 3:4]
    rs = smalls[:, 4:5]
    scale_t = smalls[:, 5:6]
    mt = smalls[:, 6:7]
    tbias = smalls[:, 7:8]
    eps_ap = cb_sb[:, 9:10]
    # padded conv activation buffers; head_in aliases r_pad (disjoint lifetime)
    y_pad = T("y_pad", [128, 130, 130], BF16)
    r_pad = T("r_pad", [128, 130, 130], BF16)
    head_in = r_pad

    from contextlib import ExitStack
    est = ExitStack()
    with tile.TileContext(nc) as tc, est:
        dram = est.enter_context(tc.tile_pool(name="dram", bufs=1, space="DRAM"))
        # DRAM scratch for x_lr (row-major fp32 for output add, bf16 for head)
        xlr32 = dram.tile([h, C, w], F32, name="xlr32")
        xlr16 = dram.tile([C, h, w], BF16, name="xlr16")

        wbf = wblk_sb.rearrange("p a b c -> p (a b c)")
        for q in range(4):
            nc.sync.dma_start(wbf[32 * q : 32 * q + 32, :],
                              d_wblk.ap()[32 * q : 32 * q + 32, :])
        nc.sync.dma_start(whead_sb.rearrange("p a b -> p (a b)"), d_whead.ap())
        nc.sync.dma_start(wtail_sb[:], d_wtail.ap())
        nc.sync.dma_start(cb_sb[:], d_cb.ap())
        nc.sync.dma_start(bn_sb[:], d_bn.ap())
        # head_in zn/zero prepack (must precede the blur-time x_lr fills)
        hflat = head_in.rearrange("p a b -> p (a b)")
        for q in range(8):
            nc.sync.dma_start(hflat[16 * q : 16 * q + 16, :],
                              d_head.ap()[16 * q : 16 * q + 16, :])

        # ================= phase 1: blur =================
        with tc.tile_pool(name="blur", bufs=1) as bp, \
             tc.tile_pool(name="blur_xp", bufs=3) as bxp, \
             tc.tile_pool(name="blur_ps", bufs=3, space="PSUM") as bps:
            wband_sb = bp.tile([128, K, 27], BF16, name="wband_sb")
            nc.sync.dma_start(wband_sb.rearrange("p a b -> p (a b)"), d_wband.ap())
            d_xp3 = d_xp.ap().rearrange("p (c t x) -> p c t x", c=C, t=NT)
            for t in range(NT):
                nt = 27 if t < 4 else 20
                xp_t = bxp.tile([128, C, 532], BF16, name="xp_t", tag="xpt")
                for q in range(8):
                    nc.sync.dma_start(xp_t[16 * q : 16 * q + 16],
                                      d_xp3[16 * q : 16 * q + 16, :, t, :])
                ps = bps.tile([27, C, w], F32, name="blur_ps_t", tag="bps")
                for kx in range(K):
                    nc.tensor.matmul(
                        ps[:, :, :],
                        lhsT=wband_sb[:, kx, :],
                        rhs=xp_t[:, :, kx : kx + 4 * w : 4],
                        start=(kx == 0),
                        stop=(kx == K - 1),
                    )
                tmp32 = bp.tile([27, C, w], F32, name="tmp32", tag="t32", bufs=2)
                tmp16 = bp.tile([27, C, w], BF16, name="tmp16", tag="t16", bufs=2)
                nc.scalar.copy(tmp32[:nt], ps[:nt])
                nc.vector.tensor_copy(tmp16[:nt], ps[:nt])
                nc.sync.dma_start(xlr32[ROWT // 4 * t : ROWT // 4 * t + nt], tmp32[:nt])
                for c in range(C):
                    nc.sync.dma_start(
                        xlr16[c, 27 * t : 27 * t + nt, :], tmp16[:nt, c, :]
                    )
                # fill head_in x_lr rows for this tile (base + shifted copy)
                r0, r1 = 27 * t, 27 * t + nt
                nc.sync.dma_start(head_in[0:3, 1 + r0 : 1 + r1, 1:129],
                                  xlr16[:, r0:r1, :])
                nc.sync.dma_start(_shifted_dst(head_in)[0:3, r0:r1, :],
                                  xlr16[:, r0:r1, :])

        # ================= phase 2+3: head conv + resblocks =================
        def conv_pass(in3, wsrc, psum_pool, evac):
            """One conv layer: 6 tap passes over 16 chunk-pairs, col-tiled x2.

            in3: [128,130,130] padded input (parts 64-127 = flat-2 copy)
            wsrc: [*, 6, 64] weight slices; evac(cp, ps) consumes psum tile
            [128, 4, 128]x2 (chunk 2cp rows in parts 0-63, 2cp+1 in 64-127).
            """
            for cp in range(16):
                rA, rB = 8 * cp, 8 * cp + 4
                ps = psum_pool.tile([128, 4, w], F32, name="cps", tag="cps")
                for p in range(6):
                    dy = p % 3 - 1
                    dx = -1 if p < 3 else 0
                    lhsT = wsrc[:, p, :]
                    nc.tensor.matmul(
                        ps[0:64], lhsT=lhsT,
                        rhs=in3[:, rA + 1 + dy : rA + 5 + dy, 1 + dx : 129 + dx],
                        start=(p == 0), stop=(p == 5),
                    )
                    nc.tensor.matmul(
                        ps[64:128], lhsT=lhsT,
                        rhs=in3[:, rB + 1 + dy : rB + 5 + dy, 1 + dx : 129 + dx],
                        start=(p == 0), stop=(p == 5),
                    )
                evac(cp, ps)

        def evac_stats(cp, ps):
            """conv1/head evac: raw -> craw (bf16) + per-chunk bn_stats."""
            for half, r0 in ((0, 8 * cp), (1, 8 * cp + 4)):
                pslc = ps[64 * half : 64 * half + 64]
                nc.scalar.activation(craw[:, r0 : r0 + 4, :], pslc, ACTF.Copy)
                nc.vector.bn_stats(
                    bnst[:, 2 * cp + half, :],
                    craw[:, r0 : r0 + 4, :].rearrange("p a b -> p (a b)"))

        def bn_allreduce(layer):
            """Reduce chunk stats, AllReduce, compute scale_t/tbias."""
            nc.vector.bn_aggr(mvloc[:], bnst.rearrange("p a b -> p (a b)"))
            # local (mean,var) -> (sum, sumsq) for cross-core reduction (all DVE)
            nc.vector.tensor_scalar_mul(stats[:, 0:1], mvloc[:, 0:1], float(h * w))
            nc.vector.tensor_mul(msq, mvloc[:, 0:1], mvloc[:, 0:1])
            nc.vector.tensor_add(sv, mvloc[:, 1:2], msq)
            nc.vector.tensor_scalar_mul(stats[:, 1:2], sv, float(h * w))
            cc_in = dram.tile([64, 2], F32, name=f"cc_in_{layer}")
            cc_out = dram.tile([64, 2], F32, name=f"cc_out_{layer}",
                               addr_space="Shared")
            nc.sync.dma_start(cc_in[:], stats)
            nc.gpsimd.collective_compute(
                "AllReduce", OP.add, replica_groups=rg,
                ins=[cc_in.opt()], outs=[cc_out.opt()],
            )
            nc.sync.dma_start(gstats, cc_out[:])
            # smalls[:,0:2] = (mean, qn) = gstats / N  (one DVE op)
            nc.vector.tensor_scalar_mul(smalls[:, 0:2], gstats, 1.0 / N_GLOBAL)
            # nvar = mean^2 - qn  (= -var); sv = sqrt(-nvar + eps)
            nc.vector.scalar_tensor_tensor(var, mean, mean, msq,
                                           op0=OP.mult, op1=OP.subtract)
            nc.scalar.activation(sv, var, ACTF.Sqrt, bias=eps_ap, scale=-1.0)
            nc.vector.reciprocal(rs, sv)
            nc.vector.tensor_mul(scale_t, rs, bn_sb[:, 2 * layer : 2 * layer + 1])
            nc.vector.tensor_mul(mt, mean, scale_t)
            nc.vector.tensor_sub(tbias, bn_sb[:, 2 * layer + 1 : 2 * layer + 2], mt)

        # ---- head conv (head_in aliases r_pad storage)
        with tc.tile_pool(name="conv_ps", bufs=8, space="PSUM") as cps_pool:
            conv_pass(head_in, whead_sb, cps_pool, evac_stats)
            bn_allreduce(0)

            # y0 = relu(bn(head)): fp32 master + bf16 padded (base + shifted)
            for buf in (y_pad, r_pad):
                nc.vector.memset(buf[:, 0, :], 0.0)      # top row
                nc.vector.memset(buf[:, 129, :], 0.0)    # bottom row
                nc.vector.memset(buf[:, 1:129, 0:1], 0.0)    # left col
                nc.vector.memset(buf[:, 1:129, 127:130], 0.0)  # right cols+guard
            for j, n in [(0, 12)] + [(12 + 16 * i, 16) for i in range(7)] + [(124, 4)]:
                nc.scalar.activation(y_pad[0:64, j + 1 : j + 1 + n, 1:129],
                                     craw[:, j : j + n, :],
                                     ACTF.Relu, bias=tbias, scale=scale_t)
                nc.vector.tensor_copy(_shifted_dst(y_pad)[:, j : j + n, :],
                                      y_pad[0:64, j + 1 : j + 1 + n, 1:129])

            # ---- 8 resblocks
            for blk in range(NB):
                conv_pass(y_pad, wblk_sb[:, 2 * blk], cps_pool, evac_stats)
                bn_allreduce(1 + blk)
                for j, n in [(0, 12)] + [(12 + 16 * i, 16) for i in range(7)] + [(124, 4)]:
                    nc.scalar.activation(r_pad[0:64, j + 1 : j + 1 + n, 1:129],
                                         craw[:, j : j + n, :],
                                         ACTF.Relu, bias=tbias, scale=scale_t)
                    nc.vector.tensor_copy(
                        _shifted_dst(r_pad)[:, j : j + n, :],
                        r_pad[0:64, j + 1 : j + 1 + n, 1:129])

                b2_ap = cb_sb[:, blk : blk + 1]

                def evac_resid(cp, ps, b2_ap=b2_ap, first=(blk == 0)):
                    for half, r0 in ((0, 8 * cp), (1, 8 * cp + 4)):
                        pslc = ps[64 * half : 64 * half + 64]
                        # block 0 reads y0 from the bf16 padded buffer (no
                        # separate fp32 y0 materialization needed)
                        prev = (y_pad[0:64, r0 + 1 : r0 + 5, 1:129] if first
                                else y_master[:, r0 : r0 + 4, :])
                        nc.vector.scalar_tensor_tensor(
                            y_master[:, r0 : r0 + 4, :], pslc, b2_ap,
                            prev, op0=OP.add, op1=OP.add,
                        )

                conv_pass(r_pad, wblk_sb[:, 2 * blk + 1], cps_pool, evac_resid)
                for j, n in [(0, 12)] + [(12 + 16 * i, 16) for i in range(7)] + [(124, 4)]:
                    nc.scalar.copy(y_pad[0:64, j + 1 : j + 1 + n, 1:129],
                                   y_master[:, j : j + n, :])
                    nc.vector.tensor_copy(
                        _shifted_dst(y_pad)[:, j : j + n, :],
                        y_pad[0:64, j + 1 : j + 1 + n, 1:129])

            # ---- tail 1x1 conv + outputs
            tb_ap = cb_sb[:, 8:9]
            # y_master is dead after the last y_pad rebuild; reuse its
            # partitions 0-2 as the x_lr fp32 staging buffer for the add
            xlr_sb = y_master[0:3, :, :]
            xlr_cij = xlr32.rearrange("i c j -> c i j")
            for c in range(C):
                nc.sync.dma_start(xlr_sb[c : c + 1], xlr_cij[c : c + 1])
            with tc.tile_pool(name="tailp", bufs=4) as tp:
                for cp in range(16):
                    ps = cps_pool.tile([128, 4, w], F32, name="tps", tag="cps")
                    for half, r0 in ((0, 8 * cp), (1, 8 * cp + 4)):
                        nc.tensor.matmul(
                            ps[64 * half : 64 * half + 3],
                            lhsT=wtail_sb[:],
                            rhs=y_pad[:, r0 + 1 : r0 + 5, 1:129],
                            start=True, stop=True,
                        )
                    for half, r0 in ((0, 8 * cp), (1, 8 * cp + 4)):
                        pslc = ps[64 * half : 64 * half + 3]
                        nstage = tp.tile([3, 4, w], F32, name="nstage", tag="nst")
                        dstage = tp.tile([3, 4, w], F32, name="dstage", tag="dst")
                        nc.scalar.activation(nstage[:], pslc, ACTF.Identity,
                                             bias=tb_ap[0:3], scale=1.0)
                        nc.vector.scalar_tensor_tensor(
                            dstage[:], pslc, tb_ap[0:3],
                            xlr_sb[:, r0 : r0 + 4, :],
                            op0=OP.add, op1=OP.add,
                        )
                        nc.sync.dma_start(d_noise.ap()[:, r0 : r0 + 4, :], nstage[:])
                        nc.sync.dma_start(d_deg.ap()[:, r0 : r0 + 4, :], dstage[:])
    return nc


_CACHE = {}


def _get_program():
    if "nc" not in _CACHE:
        nc = bacc.Bacc("TRN2", target_bir_lowering=False, debug=False,
                       num_devices=N_CORES)
        _build(nc)
        nc.compile()
        _CACHE["nc"] = nc
    return _CACHE["nc"]


def kernel(inp, zk, zn, kparams, nparams):
    kvec = _kernel_model_host(zk, kparams)  # (B, K*K)

    in_maps = [_pack_core_inputs(b, inp, zn, kvec, nparams) for b in range(B)]
    nc = _get_program()
    res = run_bass_kernel_spmd(nc, in_maps, core_ids=list(range(N_CORES)))
    kernel._last_perf = res

    deg = np.stack([res.results[b]["deg"] for b in range(B)])
    noise = np.stack([res.results[b]["noise"] for b in range(B)])
    return (deg.astype(np.float32), kvec.reshape(B, K, K),
            noise.astype(np.float32))


# revision 21
# speedup vs baseline: 1.0457x; 1.0457x over previous
"""Trainium2 Bass kernel for nn_DegModel (blind-SR degradation model).

Sharding: pure data parallel, 1 image per NeuronCore (B=8 over 8 cores).
BatchNorm batch statistics are made exact via per-layer [64,2] AllReduces.
KernelModel (1x1-conv MLP on (8,64) latents, ~1 MFLOP) runs on host in fp32.
"""
import numpy as np
import ml_dtypes

import concourse.bass as bass
import concourse.bacc as bacc
import concourse.mybir as mybir
import concourse.tile as tile
from concourse.bass_utils import run_bass_kernel_spmd

F32 = mybir.dt.float32
BF16 = mybir.dt.bfloat16
AX = mybir.AxisListType
OP = mybir.AluOpType
ACTF = mybir.ActivationFunctionType

B, C, H, W = 8, 3, 512, 512
SCALE, K = 4, 21
NF, NB = 8 * 8, 8  # 64 channels, 8 blocks
h, w = H // SCALE, W // SCALE  # 128, 128
EPS = 1e-5
PAD = K // 2  # 10
N_CORES = 8
N_GLOBAL = float(B * h * w)  # BN count over batch+spatial
ROWT = 108  # blur row-tile stride (128-row tiles, 20-row overlap)
NT = 5      # blur row tiles

bf = ml_dtypes.bfloat16


# ----------------------------------------------------------------- host math
def _np(x):
    return np.asarray(x, dtype=np.float32)


def _kernel_model_host(zk, kp):
    """Exact fp32 KernelModel: 1x1 convs + batch BN + softmax -> (B, K*K)."""
    x = _np(zk)[:, :, 0, 0]

    def conv(xx, wt, b):
        return xx @ _np(wt)[:, :, 0, 0].T + _np(b)

    def bn(xx, g, be):
        m = xx.mean(0, keepdims=True)
        v = xx.var(0, keepdims=True)
        return _np(g)[None] * (xx - m) / np.sqrt(v + EPS) + _np(be)[None]

    y = np.maximum(bn(conv(x, kp["hw"], kp["hb"]), kp["hg"], kp["hbe"]), 0)
    for blk in kp["blocks"]:
        r = np.maximum(bn(conv(y, blk["w1"], blk["b1"]), blk["g"], blk["be"]), 0)
        r = conv(r, blk["w2"], blk["b2"])
        y = y + r
    klog = conv(y, kp["tw"], kp["tb"]).astype(np.float64)
    e = np.exp(klog - klog.max(1, keepdims=True))
    return (e / e.sum(1, keepdims=True)).astype(np.float32)


def _shift2(img130):
    """flat shift by -2: out.flat[i] = in.flat[i+2]."""
    out = np.zeros_like(img130)
    out.flat[: img130.size - 2] = img130.flat[2:]
    return out


def _pack_core_inputs(b, inp, zn, kvec, npar):
    """Build the per-core input arrays (host-side layout/pack, all cheap)."""
    ins = {}
    # --- blur image: reflect pad then banded row tiles (128 rows, stride 108)
    xp = np.pad(_np(inp)[b], ((0, 0), (PAD, PAD), (PAD, PAD)), mode="reflect")
    xp_sb = np.zeros((128, C, NT, W + 2 * PAD), dtype=bf)
    for t in range(NT):
        r0 = ROWT * t
        n = min(128, xp.shape[1] - r0)
        xp_sb[:n, :, t, :] = xp[:, r0 : r0 + n, :].transpose(1, 0, 2).astype(bf)
    ins["xp_sb"] = xp_sb.reshape(128, -1)

    # --- blur band weights from kvec: wband[kx][p, di] = kvec[p-4di, kx]
    kv = kvec[b].reshape(K, K)
    wband = np.zeros((128, K, 27), dtype=np.float32)
    for kx in range(K):
        for di in range(27):
            for ky in range(K):
                p = 4 * di + ky
                if p < 128:
                    wband[p, kx, di] = kv[ky, kx]
    ins["wband"] = wband.reshape(128, -1).astype(bf)

    # --- head input prepack: zeros + zn at partitions 3..10 (and shifted copy
    #     at partitions 67..74), 130x130 zero-padded interior layout
    head = np.zeros((128, 130, 130), dtype=bf)
    znb = _np(zn)[b]  # (8, 128, 128)
    for k in range(8):
        z = np.zeros((130, 130), dtype=np.float32)
        z[1:129, 1:129] = znb[k]
        zb = z.astype(bf)
        head[3 + k] = zb
        head[64 + 3 + k] = _shift2(zb)
    ins["head_in"] = head.reshape(128, -1)

    # --- conv weights: 6 passes (3 pairs dy=-1..1 with dx=-1/+1, 3 singles dx=0)
    def pack6(wt, cin):
        out = np.zeros((128, 6, 64), dtype=bf)
        wt = _np(wt)  # (64, cin, 3, 3)
        for p in range(3):  # pairs
            out[:cin, p, :] = wt[:, :, p, 0].T.astype(bf)
            out[64 : 64 + cin, p, :] = wt[:, :, p, 2].T.astype(bf)
        for p in range(3):  # singles
            out[:cin, 3 + p, :] = wt[:, :, p, 1].T.astype(bf)
        return out

    ins["w_head"] = pack6(npar["hw"], 3 + 8).reshape(128, -1)
    wblk = np.zeros((128, 16, 6, 64), dtype=bf)
    for i, blk in enumerate(npar["blocks"]):
        wblk[:, 2 * i] = pack6(blk["w1"], 64)
        wblk[:, 2 * i + 1] = pack6(blk["w2"], 64)
    ins["w_blk"] = wblk.reshape(128, -1)
    wtail = np.zeros((128, 3), dtype=bf)
    wtail[:64, :] = _np(npar["tw"])[:, :, 0, 0].T.astype(bf)
    ins["w_tail"] = wtail

    # --- consts: col 0-7 = b2 per block, col 8 = tail bias (parts 0-2),
    #     col 9 = eps
    cb = np.zeros((64, 16), dtype=np.float32)
    for i, blk in enumerate(npar["blocks"]):
        cb[:, i] = _np(blk["b2"])
    cb[:3, 8] = _np(npar["tb"])
    cb[:, 9] = EPS
    ins["cb"] = cb

    # --- BN gamma/beta per layer (0=head, 1..8=blocks conv1)
    bn = np.zeros((64, 18), dtype=np.float32)
    bn[:, 0] = _np(npar["hg"])
    bn[:, 1] = _np(npar["hbe"])
    for i, blk in enumerate(npar["blocks"]):
        bn[:, 2 + 2 * i] = _np(blk["g"])
        bn[:, 3 + 2 * i] = _np(blk["be"])
    ins["bn"] = bn
    return ins


# -------------------------------------------------------------- device build
def _shifted_dst(tile3):
    """[64,128,128] AP over partitions 64-127 at interior flat offset -2."""
    flat = tile3.rearrange("p a b -> p (a b)")
    s = flat[64:128, 129 : 129 + 128 * 130]
    return s.rearrange("p (a b) -> p a b", b=130)[:, :, 0:128]


def _build(nc):
    d_xp = nc.dram_tensor("xp_sb", [128, C * NT * 532], BF16, kind="ExternalInput")
    d_wband = nc.dram_tensor("wband", [128, K * 27], BF16, kind="ExternalInput")
    d_head = nc.dram_tensor("head_in", [128, 130 * 130], BF16, kind="ExternalInput")
    d_whead = nc.dram_tensor("w_head", [128, 6 * 64], BF16, kind="ExternalInput")
    d_wblk = nc.dram_tensor("w_blk", [128, 16 * 6 * 64], BF16, kind="ExternalInput")
    d_wtail = nc.dram_tensor("w_tail", [128, 3], BF16, kind="ExternalInput")
    d_cb = nc.dram_tensor("cb", [64, 16], F32, kind="ExternalInput")
    d_bn = nc.dram_tensor("bn", [64, 18], F32, kind="ExternalInput")
    d_deg = nc.dram_tensor("deg", [C, h, w], F32, kind="ExternalOutput")
    d_noise = nc.dram_tensor("noise", [C, h, w], F32, kind="ExternalOutput")

    rg = [list(range(N_CORES))]

    # ---- persistent SBUF tensors (static addresses, outside the Tile pools)
    def T(name, shape, dtype):
        return nc.alloc_sbuf_tensor(name, list(shape), dtype).ap()

    wblk_sb = T("wblk_sb", [128, 16, 6, 64], BF16)
    whead_sb = T("whead_sb", [128, 6, 64], BF16)
    wtail_sb = T("wtail_sb", [128, 3], BF16)
    cb_sb = T("cb_sb", [64, 16], F32)
    bn_sb = T("bn_sb", [64, 18], F32)
    craw = T("craw", [64, h, w], BF16)
    y_master = T("y_master", [64, h, w], F32)
    bnst = T("bnst", [64, 32, 6], F32)
    mvloc = T("mvloc", [64, 2], F32)
    comm = T("comm", [64, 4], F32)
    stats = comm[:, 0:2]
    gstats = comm[:, 2:4]
    smalls = T("smalls", [64, 8], F32)
    mean = smalls[:, 0:1]
    msq = smalls[:, 1:2]
    var = smalls[:, 2:3]
    sv = smalls[:, 3:4]
    rs = smalls[:, 4:5]
    scale_t = smalls[:, 5:6]
    mt = smalls[:, 6:7]
    tbias = smalls[:, 7:8]
    eps_ap = cb_sb[:, 9:10]
    # padded conv activation buffers; head_in aliases r_pad (disjoint lifetime)
    y_pad = T("y_pad", [128, 130, 130], BF16)
    r_pad = T("r_pad", [128, 130, 130], BF16)
    head_in = r_pad

    from contextlib import ExitStack
    est = ExitStack()
    with tile.TileContext(nc) as tc, est:
        dram = est.enter_context(tc.tile_pool(name="dram", bufs=1, space="DRAM"))
        # DRAM scratch for x_lr (row-major fp32 for output add, bf16 for head)
        xlr32 = dram.tile([h, C, w], F32, name="xlr32")
        xlr16 = dram.tile([C, h, w], BF16, name="xlr16")

        # big input loads go on the Scalar engine's SWDGE queues so the
        # Sync/HWDGE queues stay free for the latency-critical blur tiles
        wbf = wblk_sb.rearrange("p a b c -> p (a b c)")
        for q in range(4):
            nc.scalar.dma_start(wbf[32 * q : 32 * q + 32, :],
                                d_wblk.ap()[32 * q : 32 * q + 32, :])
        nc.scalar.dma_start(whead_sb.rearrange("p a b -> p (a b)"), d_whead.ap())
        nc.scalar.dma_start(wtail_sb[:], d_wtail.ap())
        nc.scalar.dma_start(cb_sb[:], d_cb.ap())
        nc.scalar.dma_start(bn_sb[:], d_bn.ap())
        # head_in zn/zero prepack (must precede the blur-time x_lr fills)
        hflat = head_in.rearrange("p a b -> p (a b)")
        for q in range(8):
            nc.scalar.dma_start(hflat[16 * q : 16 * q + 16, :],
                                d_head.ap()[16 * q : 16 * q + 16, :])

        # ================= phase 1: blur =================
        with tc.tile_pool(name="blur", bufs=1) as bp, \
             tc.tile_pool(name="blur_xp", bufs=3) as bxp, \
             tc.tile_pool(name="blur_ps", bufs=3, space="PSUM") as bps:
            wband_sb = bp.tile([128, K, 27], BF16, name="wband_sb")
            nc.sync.dma_start(wband_sb.rearrange("p a b -> p (a b)"), d_wband.ap())
            d_xp3 = d_xp.ap().rearrange("p (c t x) -> p c t x", c=C, t=NT)
            for t in range(NT):
                nt = 27 if t < 4 else 20
                xp_t = bxp.tile([128, C, 532], BF16, name="xp_t", tag="xpt")
                for q in range(8):
                    nc.sync.dma_start(xp_t[16 * q : 16 * q + 16],
                                      d_xp3[16 * q : 16 * q + 16, :, t, :])
                ps = bps.tile([27, C, w], F32, name="blur_ps_t", tag="bps")
                for kx in range(K):
                    nc.tensor.matmul(
                        ps[:, :, :],
                        lhsT=wband_sb[:, kx, :],
                        rhs=xp_t[:, :, kx : kx + 4 * w : 4],
                        start=(kx == 0),
                        stop=(kx == K - 1),
                    )
                tmp32 = bp.tile([27, C, w], F32, name="tmp32", tag="t32", bufs=2)
                tmp16 = bp.tile([27, C, w], BF16, name="tmp16", tag="t16", bufs=2)
                nc.scalar.copy(tmp32[:nt], ps[:nt])
                nc.vector.tensor_copy(tmp16[:nt], ps[:nt])
                nc.sync.dma_start(xlr32[ROWT // 4 * t : ROWT // 4 * t + nt], tmp32[:nt])
                for c in range(C):
                    nc.sync.dma_start(
                        xlr16[c, 27 * t : 27 * t + nt, :], tmp16[:nt, c, :]
                    )
                # fill head_in x_lr rows for this tile (base + shifted copy)
                r0, r1 = 27 * t, 27 * t + nt
                nc.sync.dma_start(head_in[0:3, 1 + r0 : 1 + r1, 1:129],
                                  xlr16[:, r0:r1, :])
                nc.sync.dma_start(_shifted_dst(head_in)[0:3, r0:r1, :],
                                  xlr16[:, r0:r1, :])

        # ================= phase 2+3: head conv + resblocks =================
        def conv_pass(in3, wsrc, psum_pool, evac):
            """One conv layer: 6 tap passes over 16 chunk-pairs, col-tiled x2.

            in3: [128,130,130] padded input (parts 64-127 = flat-2 copy)
            wsrc: [*, 6, 64] weight slices; evac(cp, ps) consumes psum tile
            [128, 4, 128]x2 (chunk 2cp rows in parts 0-63, 2cp+1 in 64-127).
            """
            for cp in range(16):
                rA, rB = 8 * cp, 8 * cp + 4
                ps = psum_pool.tile([128, 4, w], F32, name="cps", tag="cps")
                for p in range(6):
                    dy = p % 3 - 1
                    dx = -1 if p < 3 else 0
                    lhsT = wsrc[:, p, :]
                    nc.tensor.matmul(
                        ps[0:64], lhsT=lhsT,
                        rhs=in3[:, rA + 1 + dy : rA + 5 + dy, 1 + dx : 129 + dx],
                        start=(p == 0), stop=(p == 5),
                    )
                    nc.tensor.matmul(
                        ps[64:128], lhsT=lhsT,
                        rhs=in3[:, rB + 1 + dy : rB + 5 + dy, 1 + dx : 129 + dx],
                        start=(p == 0), stop=(p == 5),
                    )
                evac(cp, ps)

        def evac_stats(cp, ps):
            """conv1/head evac: raw -> craw (bf16) + per-chunk bn_stats."""
            for half, r0 in ((0, 8 * cp), (1, 8 * cp + 4)):
                pslc = ps[64 * half : 64 * half + 64]
                nc.scalar.activation(craw[:, r0 : r0 + 4, :], pslc, ACTF.Copy)
                nc.vector.bn_stats(
                    bnst[:, 2 * cp + half, :],
                    craw[:, r0 : r0 + 4, :].rearrange("p a b -> p (a b)"))

        def bn_allreduce(layer):
            """Reduce chunk stats, AllReduce, compute scale_t/tbias."""
            nc.vector.bn_aggr(mvloc[:], bnst.rearrange("p a b -> p (a b)"))
            # local (mean,var) -> (sum, sumsq) for cross-core reduction (all DVE)
            nc.vector.tensor_scalar_mul(stats[:, 0:1], mvloc[:, 0:1], float(h * w))
            nc.vector.tensor_mul(msq, mvloc[:, 0:1], mvloc[:, 0:1])
            nc.vector.tensor_add(sv, mvloc[:, 1:2], msq)
            nc.vector.tensor_scalar_mul(stats[:, 1:2], sv, float(h * w))
            cc_in = dram.tile([64, 2], F32, name=f"cc_in_{layer}")
            cc_out = dram.tile([64, 2], F32, name=f"cc_out_{layer}",
                               addr_space="Shared")
            nc.sync.dma_start(cc_in[:], stats)
            nc.gpsimd.collective_compute(
                "AllReduce", OP.add, replica_groups=rg,
                ins=[cc_in.opt()], outs=[cc_out.opt()],
            )
            nc.sync.dma_start(gstats, cc_out[:])
            # smalls[:,0:2] = (mean, qn) = gstats / N  (one DVE op)
            nc.vector.tensor_scalar_mul(smalls[:, 0:2], gstats, 1.0 / N_GLOBAL)
            # nvar = mean^2 - qn  (= -var); sv = sqrt(-nvar + eps)
            nc.vector.scalar_tensor_tensor(var, mean, mean, msq,
                                           op0=OP.mult, op1=OP.subtract)
            nc.scalar.activation(sv, var, ACTF.Sqrt, bias=eps_ap, scale=-1.0)
            nc.vector.reciprocal(rs, sv)
            nc.vector.tensor_mul(scale_t, rs, bn_sb[:, 2 * layer : 2 * layer + 1])
            nc.vector.tensor_mul(mt, mean, scale_t)
            nc.vector.tensor_sub(tbias, bn_sb[:, 2 * layer + 1 : 2 * layer + 2], mt)

        # ---- head conv (head_in aliases r_pad storage)
        with tc.tile_pool(name="conv_ps", bufs=8, space="PSUM") as cps_pool:
            conv_pass(head_in, whead_sb, cps_pool, evac_stats)
            bn_allreduce(0)

            # y0 = relu(bn(head)): fp32 master + bf16 padded (base + shifted)
            for buf in (y_pad, r_pad):
                nc.vector.memset(buf[:, 0, :], 0.0)      # top row
                nc.vector.memset(buf[:, 129, :], 0.0)    # bottom row
                nc.vector.memset(buf[:, 1:129, 0:1], 0.0)    # left col
                nc.vector.memset(buf[:, 1:129, 127:130], 0.0)  # right cols+guard
            for j, n in [(0, 12)] + [(12 + 16 * i, 16) for i in range(7)] + [(124, 4)]:
                nc.scalar.activation(y_pad[0:64, j + 1 : j + 1 + n, 1:129],
                                     craw[:, j : j + n, :],
                                     ACTF.Relu, bias=tbias, scale=scale_t)
                nc.vector.tensor_copy(_shifted_dst(y_pad)[:, j : j + n, :],
                                      y_pad[0:64, j + 1 : j + 1 + n, 1:129])

            # ---- 8 resblocks
            for blk in range(NB):
                conv_pass(y_pad, wblk_sb[:, 2 * blk], cps_pool, evac_stats)
                bn_allreduce(1 + blk)
                for j, n in [(0, 12)] + [(12 + 16 * i, 16) for i in range(7)] + [(124, 4)]:
                    nc.scalar.activation(r_pad[0:64, j + 1 : j + 1 + n, 1:129],
                                         craw[:, j : j + n, :],
                                         ACTF.Relu, bias=tbias, scale=scale_t)
                    nc.vector.tensor_copy(
                        _shifted_dst(r_pad)[:, j : j + n, :],
                        r_pad[0:64, j + 1 : j + 1 + n, 1:129])

                b2_ap = cb_sb[:, blk : blk + 1]

                def evac_resid(cp, ps, b2_ap=b2_ap, first=(blk == 0)):
                    for half, r0 in ((0, 8 * cp), (1, 8 * cp + 4)):
                        pslc = ps[64 * half : 64 * half + 64]
                        # block 0 reads y0 from the bf16 padded buffer (no
                        # separate fp32 y0 materialization needed)
                        prev = (y_pad[0:64, r0 + 1 : r0 + 5, 1:129] if first
                                else y_master[:, r0 : r0 + 4, :])
                        nc.vector.scalar_tensor_tensor(
                            y_master[:, r0 : r0 + 4, :], pslc, b2_ap,
                            prev, op0=OP.add, op1=OP.add,
                        )

                conv_pass(r_pad, wblk_sb[:, 2 * blk + 1], cps_pool, evac_resid)
                for j, n in [(0, 12)] + [(12 + 16 * i, 16) for i in range(7)] + [(124, 4)]:
                    nc.scalar.copy(y_pad[0:64, j + 1 : j + 1 + n, 1:129],
                                   y_master[:, j : j + n, :])
                    nc.vector.tensor_copy(
                        _shifted_dst(y_pad)[:, j : j + n, :],
                        y_pad[0:64, j + 1 : j + 1 + n, 1:129])

            # ---- tail 1x1 conv + outputs
            tb_ap = cb_sb[:, 8:9]
            # y_master is dead after the last y_pad rebuild; reuse its
            # partitions 0-2 as the x_lr fp32 staging buffer for the add
            xlr_sb = y_master[0:3, :, :]
            xlr_cij = xlr32.rearrange("i c j -> c i j")
            for c in range(C):
                nc.sync.dma_start(xlr_sb[c : c + 1], xlr_cij[c : c + 1])
            with tc.tile_pool(name="tailp", bufs=4) as tp:
                for cp in range(16):
                    ps = cps_pool.tile([128, 4, w], F32, name="tps", tag="cps")
                    for half, r0 in ((0, 8 * cp), (1, 8 * cp + 4)):
                        nc.tensor.matmul(
                            ps[64 * half : 64 * half + 3],
                            lhsT=wtail_sb[:],
                            rhs=y_pad[:, r0 + 1 : r0 + 5, 1:129],
                            start=True, stop=True,
                        )
                    for half, r0 in ((0, 8 * cp), (1, 8 * cp + 4)):
                        pslc = ps[64 * half : 64 * half + 3]
                        nstage = tp.tile([3, 4, w], F32, name="nstage", tag="nst")
                        dstage = tp.tile([3, 4, w], F32, name="dstage", tag="dst")
                        nc.scalar.activation(nstage[:], pslc, ACTF.Identity,
                                             bias=tb_ap[0:3], scale=1.0)
                        nc.vector.scalar_tensor_tensor(
                            dstage[:], pslc, tb_ap[0:3],
                            xlr_sb[:, r0 : r0 + 4, :],
                            op0=OP.add, op1=OP.add,
                        )
                        nc.sync.dma_start(d_noise.ap()[:, r0 : r0 + 4, :], nstage[:])
                        nc.sync.dma_start(d_deg.ap()[:, r0 : r0 + 4, :], dstage[:])
    return nc


_CACHE = {}


def _get_program():
    if "nc" not in _CACHE:
        nc = bacc.Bacc("TRN2", target_bir_lowering=False, debug=False,
                       num_devices=N_CORES)
        _build(nc)
        nc.compile()
        _CACHE["nc"] = nc
    return _CACHE["nc"]


def kernel(inp, zk, zn, kparams, nparams):
    kvec = _kernel_model_host(zk, kparams)  # (B, K*K)

    in_maps = [_pack_core_inputs(b, inp, zn, kvec, nparams) for b in range(B)]
    nc = _get_program()
    res = run_bass_kernel_spmd(nc, in_maps, core_ids=list(range(N_CORES)))
    kernel._last_perf = res

    deg = np.stack([res.results[b]["deg"] for b in range(B)])
    noise = np.stack([res.results[b]["noise"] for b in range(B)])
    return (deg.astype(np.float32), kvec.reshape(B, K, K),
            noise.astype(np.float32))
